# revision 28
# baseline (speedup 1.0000x reference)
"""GQA attention kernel for 8 Trainium2 NeuronCores.

Sharding: tensor-parallel over heads. Core i handles query heads (2i, 2i+1)
and KV head i//2. Out-proj is row-parallel: each core emits a partial
[S, DIM] output; the host sums the 8 partials and adds the output bias.

bf16 datapath (PSUM accumulation in fp32), fused per-512-chunk pipeline:
projection passes, the previous chunk's out-projection (as PE filler for
bias/RoPE latency), and causal attention all interleave so the tensor
engine stays busy and DMA streams hide underneath. Diagonal score strips
only compute the valid query sub-range; the causal mask is a single
128x128 0/1 triangle applied multiplicatively on the GpSimd engine.

On-chip layouts keep head_dim (128) on partitions and sequence on the free
axis, so QK^T needs no transposes, softmax statistics are PE ones-matmuls,
and the attention weights feed the AV matmul directly from the exp output.
"""

import numpy as np
import ml_dtypes

BF16 = ml_dtypes.bfloat16

DIM = 2048
Q_HEADS = 16
KV_HEADS = 4
HEAD_DIM = 128
S = 2048
MAX_LEN = 2048
ROPE_THETA = 10000.0
ROPE_FACTOR = 8.0
N_CORES = 8
SCALE = 1.0 / np.sqrt(HEAD_DIM)

_F32R_CACHE = {}


def _rope_cos_sin_T():
    d = HEAD_DIM
    seq_eff = max(S, MAX_LEN)
    base_adj = (ROPE_FACTOR * seq_eff / MAX_LEN - (ROPE_FACTOR - 1.0)) ** (d / (d - 2))
    adjusted_base = ROPE_THETA * base_adj
    inv_freq = 1.0 / adjusted_base ** (np.arange(0, d, 2, dtype=np.float32) / d)
    pos = np.arange(S, dtype=np.float32)
    freqs = pos[:, None] * inv_freq[None, :]
    emb = np.concatenate([freqs, freqs], axis=-1)  # [S, d]
    return (
        np.ascontiguousarray(np.cos(emb).T.astype(np.float32)),  # [d, S]
        np.ascontiguousarray(np.sin(emb).T.astype(np.float32)),
    )


def _tri01():
    # multiplicative causal mask for the 128x128 block on the diagonal:
    # keep k <= q, zero the k > q triangle. Same for every diagonal strip.
    k = np.arange(128)[:, None]
    q = np.arange(128)[None, :]
    return np.ascontiguousarray(np.where(k > q, 0.0, 1.0).astype(BF16))


def _build_program():
    import concourse.bass as bass
    import concourse.tile as tile
    from concourse import mybir
    import bass_rust
    from concourse.vector_clock import ScopedClock
    from concourse.masks import make_identity

    # --- workaround: walrus CTRL instructions accept a single sync wait;
    # split the TileContext end-drain waits across one SP nop each.
    def _patched_drain_and_barrier(self, tick_clock, wait_clock):
        nop0 = self.nc.sync.nop(nofuse=True)
        wait_clock.add_sem_waits(nop0.ins, ScopedClock({None: tick_clock.global_clock}))
        si = nop0.ins.sync_info
        ws = list(si.on_wait) if si is not None else []
        if len(ws) > 1:
            nop0.ins.sync_info = bass_rust.SyncInfo(
                on_wait=ws[:1], on_update=list(si.on_update))
            for i in range(1, len(ws)):
                nop = self.nc.sync.nop(nofuse=True)
                nop.ins.sync_info = bass_rust.SyncInfo(on_wait=ws[i:i + 1], on_update=[])
        self.nc.sync.drain()
        self.nc.all_engine_barrier()
        popped = self.nc._tile_sem_poison_stack.pop()
        assert popped is self._sem_poison
        self.nc.clear_and_free_semaphores(list(self.sems.allocated().values()))
        self.nc.all_engine_barrier()

    tile.TileContext._drain_and_barrier = _patched_drain_and_barrier

    def _split_multi_waits(nc):
        # this walrus build accepts a single sync-wait slot on several
        # instruction encodings; peel extra waits onto same-engine NoOps.
        cnt = 0
        for f in nc.m.functions:
            for bb in f.blocks:
                new_l = []
                for inst in bb.instructions:
                    si = inst.sync_info
                    ws = list(si.on_wait) if si is not None else []
                    if len(ws) > 1:
                        for w in ws[:-1]:
                            nop = mybir.InstNoOp(
                                name=f"{inst.name}_wsplit{cnt}", engine=inst.engine,
                                bass_nofuse=True,
                                sync_info=mybir.SyncInfo(on_wait=[w], on_update=[]))
                            nc.register_instruction(nop, overwrite=True)
                            new_l.append(nop)
                            cnt += 1
                        inst.sync_info = mybir.SyncInfo(
                            on_wait=[ws[-1]], on_update=list(si.on_update))
                    new_l.append(inst)
                bb.instructions = new_l

    f32 = mybir.dt.float32
    bf16 = mybir.dt.bfloat16
    AF = mybir.ActivationFunctionType
    OP = mybir.AluOpType

    nc = bass.Bass()
    qT_in = nc.dram_tensor("queryT", [DIM, S], bf16, kind="ExternalInput")
    kT_in = nc.dram_tensor("keyT", [DIM, S], bf16, kind="ExternalInput")
    vT_in = nc.dram_tensor("valueT", [DIM, S], bf16, kind="ExternalInput")
    # weights are host-prepacked so each of the 128 SBUF partition lines is
    # one contiguous DRAM row (single big DMA descriptor per partition)
    wq_in = nc.dram_tensor("wqP", [128, 16 * 256], bf16, kind="ExternalInput")
    wk_in = nc.dram_tensor("wkP", [128, 16 * 128], bf16, kind="ExternalInput")
    wv_in = nc.dram_tensor("wvP", [128, 16 * 128], bf16, kind="ExternalInput")
    wo_in = nc.dram_tensor("woP", [128, 2 * DIM], bf16, kind="ExternalInput")
    bq_in = nc.dram_tensor("bq_col", [128, 2], f32, kind="ExternalInput")
    bk_in = nc.dram_tensor("bk_col", [128, 1], f32, kind="ExternalInput")
    bv_in = nc.dram_tensor("bv_col", [128, 1], f32, kind="ExternalInput")
    cos_in = nc.dram_tensor("cosT", [128, S], bf16, kind="ExternalInput")
    sin_in = nc.dram_tensor("sinT", [128, S], bf16, kind="ExternalInput")
    tri_in = nc.dram_tensor("tri01", [128, 128], bf16, kind="ExternalInput")
    out_dram = nc.dram_tensor("partial", [S, DIM], bf16, kind="ExternalOutput")

    qT_r = qT_in.rearrange("(co ci) s -> ci co s", ci=128)
    kT_r = kT_in.rearrange("(co ci) s -> ci co s", ci=128)
    vT_r = vT_in.rearrange("(co ci) s -> ci co s", ci=128)

    with tile.TileContext(nc) as tc:
        with (
            tc.tile_pool(name="const", bufs=1) as cpool,
            tc.tile_pool(name="streamq", bufs=6) as spq,
            tc.tile_pool(name="streamkv", bufs=5) as spool,
            tc.tile_pool(name="work", bufs=2) as wpool,
            tc.tile_pool(name="outb", bufs=4) as opool,
            tc.tile_pool(name="acts", bufs=1) as apool,
            tc.tile_pool(name="attnw", bufs=1) as atpool,
            tc.tile_pool(name="ps_proj", bufs=1, space="PSUM") as psp,
            tc.tile_pool(name="ps_attn", bufs=2, space="PSUM") as psa,
            tc.tile_pool(name="ps_acc", bufs=1, space="PSUM") as psc,
        ):
            # ---- constants / weights: issued on the Activation HWDGE queue
            # so they overlap the activation streams on the sync queue.
            # Ordered by first use: wq (pass1), rope tables + biases, wk/wv
            # (pass2), tri (attention), wo (out-proj, first needed at sc=1).
            # wq lands in two pieces so the first projection matmul only
            # waits on a 128 KB transfer instead of the full megabyte
            wq_v = wq_in.rearrange("p (co d) -> p co d", co=16)
            wq_sb = cpool.tile([128, 16, 256], bf16)
            nc.scalar.dma_start(wq_sb[:, 0:4], wq_v[:, 0:4])
            bq_sb = cpool.tile([128, 2], f32)
            nc.scalar.dma_start(bq_sb[:], bq_in[:])
            nc.scalar.dma_start(wq_sb[:, 4:16], wq_v[:, 4:16])
            bk_sb = cpool.tile([128, 1], f32)
            nc.scalar.dma_start(bk_sb[:], bk_in[:])
            bv_sb = cpool.tile([128, 1], f32)
            nc.scalar.dma_start(bv_sb[:], bv_in[:])
            cos_sb = cpool.tile([128, S], bf16)
            nc.scalar.dma_start(cos_sb[:], cos_in[:])
            sin_sb = cpool.tile([128, S], bf16)
            nc.scalar.dma_start(sin_sb[:], sin_in[:])
            wk_sb = cpool.tile([128, 16, 128], bf16)
            nc.scalar.dma_start(wk_sb[:], wk_in.rearrange("p (co d) -> p co d", co=16))
            wv_sb = cpool.tile([128, 16, 128], bf16)
            nc.scalar.dma_start(wv_sb[:], wv_in.rearrange("p (co d) -> p co d", co=16))
            tri_sb = cpool.tile([128, 128], bf16)
            nc.scalar.dma_start(tri_sb[:], tri_in[:])
            wo_sb = cpool.tile([128, 2, DIM], bf16)
            nc.scalar.dma_start(wo_sb[:], wo_in.rearrange("p (h e) -> p h e", h=2))
            ones_f = cpool.tile([128, 128], f32)
            nc.vector.memset(ones_f[:], 1.0)
            ones_mat = cpool.tile([128, 128], bf16)
            nc.vector.tensor_copy(out=ones_mat[:], in_=ones_f[:])
            ident = cpool.tile([128, 128], bf16)
            make_identity(nc, ident[:])

            # ---- persistent activations
            q_rot = [apool.tile([128, S], bf16, tag=f"qrot{h}", name=f"qrot{h}")
                     for h in range(2)]
            k_rot = apool.tile([128, S], bf16, tag="krot")
            v_sb = apool.tile([128, S], bf16, tag="vsb")   # [k_local, (kt d)] rows
            ctxT = [apool.tile([128, S], bf16, tag=f"ctx{h}", name=f"ctx{h}")
                    for h in range(2)]

            def rope(dst, raw, sc):
                # dst = raw*cos + swap(raw)*sinMod, sinMod has the -1 on the
                # low half baked in host-side (rotate_half sign).
                ssl = slice(sc * 512, sc * 512 + 512)
                swp = wpool.tile([128, 512], bf16, tag="ropeswp")
                nc.vector.tensor_copy(out=swp[0:64, :], in_=raw[64:128, :])
                nc.vector.tensor_copy(out=swp[64:128, :], in_=raw[0:64, :])
                tmp = wpool.tile([128, 512], bf16, tag="ropetmp")
                nc.vector.tensor_tensor(tmp[:], swp[:], sin_sb[:, ssl], OP.mult)
                nc.vector.tensor_tensor(dst[:, ssl], raw[:], cos_sb[:, ssl], OP.mult)
                nc.vector.tensor_tensor(dst[:, ssl], dst[:, ssl], tmp[:], OP.add)

            def outproj_st(qc, st):
                # partial[qc chunk, st 128-row block] = sum_h ctxT_h.T @ woT_h
                stsl = slice(qc * 512 + st * 128, qc * 512 + st * 128 + 128)
                for ecp in range(2):
                    e0 = slice(ecp * 1024, ecp * 1024 + 512)
                    e1 = slice(ecp * 1024 + 512, ecp * 1024 + 1024)
                    po0 = psc.tile([128, 512], f32, tag="psum")
                    po1 = psc.tile([128, 512], f32, tag="pctx")
                    # h-outer so the ctx stationary is reused across ec
                    nc.tensor.matmul(po0[:], ctxT[0][:, stsl],
                                     wo_sb[:, 0, e0], start=True, stop=False)
                    nc.tensor.matmul(po1[:], ctxT[0][:, stsl],
                                     wo_sb[:, 0, e1], start=True, stop=False)
                    nc.tensor.matmul(po0[:], ctxT[1][:, stsl],
                                     wo_sb[:, 1, e0], start=False, stop=True)
                    nc.tensor.matmul(po1[:], ctxT[1][:, stsl],
                                     wo_sb[:, 1, e1], start=False, stop=True)
                    ot = opool.tile([128, 1024], bf16, tag="ot")
                    nc.vector.tensor_copy(out=ot[:, 0:512], in_=po0[:])
                    nc.vector.tensor_copy(out=ot[:, 512:1024], in_=po1[:])
                    nc.sync.dma_start(
                        out_dram[stsl, ecp * 1024:ecp * 1024 + 1024], ot[:])

            for sc in range(4):
                ssl = slice(sc * 512, sc * 512 + 512)
                # ---- stream this chunk of q/k/v (4 cc per DMA)
                # qmap[cc] -> (tile, local index). The first chunk's leading
                # q tiles are split small so the very first matmul starts as
                # soon as 128 KB has landed.
                qmap, kts, vts = [], [], []
                if sc == 0:
                    qa = cpool.tile([128, 1, 512], bf16)
                    nc.sync.dma_start(qa[:], qT_r[:, 0:1, ssl])
                    qb = cpool.tile([128, 3, 512], bf16)
                    nc.sync.dma_start(qb[:], qT_r[:, 1:4, ssl])
                    qmap += [(qa, 0), (qb, 0), (qb, 1), (qb, 2)]
                    bs = range(1, 4)
                else:
                    bs = range(4)
                for b in bs:
                    qt = spq.tile([128, 4, 512], bf16, tag="qs")
                    nc.sync.dma_start(qt[:], qT_r[:, 4 * b:4 * b + 4, ssl])
                    qmap += [(qt, j) for j in range(4)]
                for b in range(4):
                    kt_ = spool.tile([128, 4, 512], bf16, tag="ks")
                    nc.sync.dma_start(kt_[:], kT_r[:, 4 * b:4 * b + 4, ssl])
                    kts.append(kt_)
                for b in range(4):
                    vt = spool.tile([128, 4, 512], bf16, tag="vs")
                    nc.sync.dma_start(vt[:], vT_r[:, 4 * b:4 * b + 4, ssl])
                    vts.append(vt)

                # ---- projection pass 1: the two q heads
                pq0 = psp.tile([128, 512], f32, tag="A")
                pq1 = psp.tile([128, 512], f32, tag="B")
                for cc in range(16):
                    st_, sp_ = cc == 0, cc == 15
                    qt_, qj_ = qmap[cc]
                    mv = qt_[:, qj_]
                    nc.tensor.matmul(pq0[:], wq_sb[:, cc, 0:128], mv,
                                     start=st_, stop=sp_)
                    nc.tensor.matmul(pq1[:], wq_sb[:, cc, 128:256], mv,
                                     start=st_, stop=sp_)
                q0_raw = wpool.tile([128, 512], bf16, tag="raw")
                nc.scalar.activation(q0_raw[:], pq0[:], AF.Identity, bias=bq_sb[:, 0:1])
                q1_raw = wpool.tile([128, 512], bf16, tag="raw")
                nc.scalar.activation(q1_raw[:], pq1[:], AF.Identity, bias=bq_sb[:, 1:2])

                # out-proj first half: PE filler while the q biases drain
                if sc > 0:
                    outproj_st(sc - 1, 0)
                    outproj_st(sc - 1, 1)
                rope(q_rot[0], q0_raw, sc)
                rope(q_rot[1], q1_raw, sc)

                # ---- projection pass 2: k then v, reusing banks A/B
                pk = psp.tile([128, 512], f32, tag="A")
                for cc in range(16):
                    nc.tensor.matmul(pk[:], wk_sb[:, cc], kts[cc // 4][:, cc % 4],
                                     start=cc == 0, stop=cc == 15)
                pv = psp.tile([128, 512], f32, tag="B")
                for cc in range(16):
                    nc.tensor.matmul(pv[:], wv_sb[:, cc], vts[cc // 4][:, cc % 4],
                                     start=cc == 0, stop=cc == 15)
                k_raw = wpool.tile([128, 512], bf16, tag="raw")
                nc.scalar.activation(k_raw[:], pk[:], AF.Identity, bias=bk_sb[:])
                v_raw = wpool.tile([128, 512], bf16, tag="raw")
                nc.scalar.activation(v_raw[:], pv[:], AF.Identity, bias=bv_sb[:])

                # out-proj second half: covers the k bias + rope latency
                if sc > 0:
                    outproj_st(sc - 1, 2)
                    outproj_st(sc - 1, 3)
                rope(k_rot, k_raw, sc)

                # v transpose: 4 blocks into one bank, one copy out
                vtr = psp.tile([128, 512], bf16, tag="A")
                for j in range(4):
                    nc.tensor.transpose(vtr[:, j * 128:(j + 1) * 128],
                                        v_raw[:, j * 128:(j + 1) * 128], ident[:])
                nc.vector.tensor_copy(out=v_sb[:, ssl], in_=vtr[:])

                # ---- attention for this query chunk, both heads.
                # Score strips are processed in pairs sharing one 2-bank
                # PSUM tile so each exp covers [128, 1024] (halves the
                # per-op Activation overhead). Diagonal strips compute the
                # full query range (the pre-diagonal columns are never read
                # downstream); the within-block k>q triangle is zeroed
                # multiplicatively on GpSimd after the exp, and the ones/AV
                # accumulations below still use the valid sub-range only.
                n_kt = 4 * (sc + 1)
                at = [atpool.tile([128, 16, 512], bf16, tag=f"at{h}",
                                  name=f"at{h}_{sc}") for h in range(2)]
                for kp in range(n_kt // 2):
                    for h in range(2):
                        pst = psa.tile([128, 2, 512], f32, tag="pst")
                        for j in range(2):
                            kt = 2 * kp + j
                            nc.tensor.matmul(
                                pst[:, j], k_rot[:, kt * 128:(kt + 1) * 128],
                                q_rot[h][:, ssl], start=True, stop=True)
                        nc.scalar.activation(at[h][:, 2 * kp:2 * kp + 2],
                                             pst[:], AF.Exp,
                                             scale=float(SCALE))
                        for j in range(2):
                            kt = 2 * kp + j
                            r = kt - 4 * sc
                            if r >= 0:
                                qlo = 128 * r
                                nc.gpsimd.tensor_tensor(
                                    at[h][:, kt, qlo:qlo + 128],
                                    at[h][:, kt, qlo:qlo + 128], tri_sb[:],
                                    OP.mult)
                for h in range(2):
                    psum = psc.tile([128, 512], f32, tag="psum")
                    for kt in range(n_kt):
                        r = kt - 4 * sc
                        qlo = 128 * r if r > 0 else 0
                        nc.tensor.matmul(psum[:, qlo:512], ones_mat[:],
                                         at[h][:, kt, qlo:512],
                                         start=kt == 0, stop=kt == n_kt - 1,
                                         skip_group_check=True)
                    # softmax normalizer as exp(-ln(sum)) on the Activation
                    # engine: reads PSUM directly (frees the bank), and keeps
                    # the slow DVE reciprocal off the context critical path
                    lden = wpool.tile([128, 512], f32, tag="lden")
                    nc.scalar.activation(lden[:], psum[:], AF.Ln)
                    pctx = psc.tile([128, 512], f32, tag="pctx")
                    for kt in range(n_kt):
                        r = kt - 4 * sc
                        qlo = 128 * r if r > 0 else 0
                        nc.tensor.matmul(pctx[:, qlo:512],
                                         v_sb[:, kt * 128:(kt + 1) * 128],
                                         at[h][:, kt, qlo:512],
                                         start=kt == 0, stop=kt == n_kt - 1,
                                         skip_group_check=True)
                    bc = wpool.tile([128, 512], f32, tag="bc")
                    nc.scalar.activation(bc[:], lden[:], AF.Exp, scale=-1.0)
                    # normalize per 128-query block so the out-projection of
                    # this chunk can start before the whole chunk is done
                    for stq in range(4):
                        s0 = stq * 128
                        nc.vector.tensor_tensor(
                            ctxT[h][:, sc * 512 + s0:sc * 512 + s0 + 128],
                            pctx[:, s0:s0 + 128], bc[:, s0:s0 + 128], OP.mult)

            for st in range(4):
                outproj_st(3, st)
    _split_multi_waits(nc)
    return nc


def kernel(query, key, value, Wq, bq, Wk, bk, Wv, bv, Wo, bo):
    from concourse.bass_utils import run_bass_kernel_spmd

    query = np.asarray(query, np.float32)
    key = np.asarray(key, np.float32)
    value = np.asarray(value, np.float32)
    B = query.shape[0]
    qT = np.ascontiguousarray(query.reshape(S, DIM).T.astype(BF16))
    kT = np.ascontiguousarray(key.reshape(S, DIM).T.astype(BF16))
    vT = np.ascontiguousarray(value.reshape(S, DIM).T.astype(BF16))
    cosT, sinT = _rope_cos_sin_T()
    sinT = sinT.copy()
    sinT[0:64, :] *= -1.0  # rotate_half: low half gets -x2*sin
    cosT = np.ascontiguousarray(cosT.astype(BF16))
    sinT = np.ascontiguousarray(sinT.astype(BF16))
    tri = _tri01()

    if "nc" not in _F32R_CACHE:
        _F32R_CACHE["nc"] = _build_program()
    nc = _F32R_CACHE["nc"]

    in_maps = []
    for i in range(N_CORES):
        g = i // 2
        # SBUF-packed weights: row p = partition line, i.e. wq_sb[p, co, d]
        # flattened — wqT[co*128+p, d] with wqT = Wq_slice.T [DIM, cols]
        Wq_s = np.asarray(Wq, np.float32)[256 * i:256 * (i + 1), :].T.astype(BF16)
        Wq_p = np.ascontiguousarray(
            Wq_s.reshape(16, 128, 256).transpose(1, 0, 2).reshape(128, 16 * 256))
        Wk_s = np.asarray(Wk, np.float32)[128 * g:128 * (g + 1), :].T.astype(BF16)
        Wk_p = np.ascontiguousarray(
            Wk_s.reshape(16, 128, 128).transpose(1, 0, 2).reshape(128, 16 * 128))
        Wv_s = np.asarray(Wv, np.float32)[128 * g:128 * (g + 1), :].T.astype(BF16)
        Wv_p = np.ascontiguousarray(
            Wv_s.reshape(16, 128, 128).transpose(1, 0, 2).reshape(128, 16 * 128))
        Wo_s = np.asarray(Wo, np.float32)[:, 256 * i:256 * (i + 1)].T.astype(BF16)
        Wo_p = np.ascontiguousarray(
            Wo_s.reshape(2, 128, DIM).transpose(1, 0, 2).reshape(128, 2 * DIM))
        bq_c = np.ascontiguousarray(
            np.asarray(bq, np.float32)[256 * i:256 * (i + 1)].reshape(2, 128).T)
        bk_c = np.asarray(bk, np.float32)[128 * g:128 * (g + 1)].reshape(128, 1)
        bv_c = np.asarray(bv, np.float32)[128 * g:128 * (g + 1)].reshape(128, 1)
        in_maps.append({
            "queryT": qT, "keyT": kT, "valueT": vT,
            "wqP": Wq_p, "wkP": Wk_p, "wvP": Wv_p, "woP": Wo_p,
            "bq_col": bq_c, "bk_col": np.ascontiguousarray(bk_c),
            "bv_col": np.ascontiguousarray(bv_c),
            "cosT": cosT, "sinT": sinT, "tri01": tri,
        })

    _F32R_CACHE["in_maps"] = in_maps
    globals()["_LAST_IN_MAPS"] = in_maps
    res = run_bass_kernel_spmd(nc, in_maps, list(range(N_CORES)))
    out = np.asarray(res.results[0]["partial"]).astype(np.float32)
    for i in range(1, N_CORES):
        out = out + np.asarray(res.results[i]["partial"]).astype(np.float32)
    out = out + np.asarray(bo, np.float32)[None, :]
    return out.reshape(B, S, DIM).astype(np.float32)
